# revision 3
# baseline (speedup 1.0000x reference)
"""CosineRouter Trainium2 kernel.

Computes, for x:(64,2048,1024) f32, W:(256,1024), b:(256,), centers:(512,256):
  xp   = x @ W.T + b                      (projection to expert_dim=256)
  cos  = l2norm(xp) @ l2norm(centers).T   (cosine sim vs 512 experts)
  topk_vals, topk_idx = top_k(cos, 2); topk_probs = softmax(topk_vals)
Returns (topk_probs f32 (64,2048,2), topk_idx int32 (64,2048,2)).

Distribution: data-parallel over tokens across 8 NeuronCores; W/b/centers
replicated. On-device layout keeps expert_dim on partitions for the
projection (xp[d, tok]) so the cosine matmul consumes it directly as the
stationary operand, producing sim[tok, expert] whose top-2 is found with
the vector engine's max8/max_index instructions. The l2-normalization of
xp is deferred: top-2 selection is scale-invariant per token, so only the
two selected values are rescaled by 1/||xp|| (computed via a ones-column
matmul over xp^2), and softmax(2) is evaluated with the exp/ln table set.

Matmul precision modes:
  fp32  — native fp32 matmuls (4 cycles/row on the PE).
  bf16c — error-compensated bf16: each operand split into hi+lo bf16
          (hi = bf16(v), lo = bf16(v - hi)); product evaluated as
          hi*hi + hi*lo + lo*hi, three 1-cycle/row passes accumulated in
          fp32 PSUM. ~2^-17 relative error vs fp32's 2^-24 — small enough
          to preserve top-2 ordering (2 flipped tokens out of 131072 on
          the reference inputs). x is split host-side (same bytes moved);
          xp is split on-chip during PSUM eviction.
"""

import os
import sys

if "/opt/trn_rl_repo" not in sys.path:
    sys.path.insert(0, "/opt/trn_rl_repo")

import ml_dtypes
import numpy as np

import concourse.bacc as bacc
import concourse.mybir as mybir
import concourse.tile as tile
from concourse.bass_utils import run_bass_kernel_spmd

# Problem shapes (hardcoded per contract)
BS, NTOK, IN_DIM = 64, 2048, 1024
ED, NE = 256, 512  # expert_dim, n_experts
NCORES = 8
TOKS = BS * NTOK  # 131072
TPC = TOKS // NCORES  # 16384 tokens per core
WIN = 1024  # tokens per window
NWIN = TPC // WIN  # 16
NKC = IN_DIM // 128  # 8 contraction chunks
NG = WIN // 128  # 8 token groups of 128 per window

MODE = os.environ.get("KERNEL_MODE", "bf16c")  # "fp32" | "bf16c"

F32 = mybir.dt.float32
BF16 = mybir.dt.bfloat16
I32 = mybir.dt.int32
U32 = mybir.dt.uint32
AF = mybir.ActivationFunctionType
BF = ml_dtypes.bfloat16


def _common_epilogue(nc, tc, pools, win, xp2_sb, one_sb, sim_maker, probs_d, idx_d):
    """Per-window: top-2 + sum-of-squares + softmax + output DMA."""
    small_p, out_p, sim_p, ps_sim, ps_ss = pools
    pss = ps_ss.tile([128, NG], F32, tag="pss")
    vals = small_p.tile([128, NG, 8], F32, tag="vals")
    idxs = small_p.tile([128, NG, 8], U32, tag="idxs")
    for g in range(NG):
        gs = slice(g * 128, (g + 1) * 128)
        psim = ps_sim.tile([128, NE], F32, tag="psim")
        sim_maker(psim, gs)
        sim = sim_p.tile([128, NE], F32, tag="sim")
        nc.scalar.activation(sim[:], psim[:], AF.Copy)
        nc.vector.max(out=vals[:, g, :], in_=sim[:])
        nc.vector.max_index(out=idxs[:, g, :], in_max=vals[:, g, :], in_values=sim[:])
        for j in range(2):
            nc.tensor.matmul(
                pss[:, g : g + 1], xp2_sb[j][:, gs], one_sb[:],
                start=(j == 0), stop=(j == 1),
            )

    # inv_norm = exp(-0.5*ln(ss)); softmax over the two selected vals
    lss = small_p.tile([128, NG], F32, tag="lss")
    einv = small_p.tile([128, NG], F32, tag="einv")
    nc.scalar.activation(lss[:], pss[:], AF.Ln)
    nc.scalar.activation(einv[:], lss[:], AF.Exp, scale=-0.5)

    d21 = small_p.tile([128, NG], F32, tag="d21")
    tlog = small_p.tile([128, NG], F32, tag="tlog")
    q2 = small_p.tile([128, NG], F32, tag="q2")
    s12 = small_p.tile([128, NG], F32, tag="s12")
    pout = out_p.tile([128, NG, 2], F32, tag="pout")
    iout = out_p.tile([128, NG, 2], I32, tag="iout")
    nc.vector.tensor_sub(d21[:], vals[:, :, 1], vals[:, :, 0])
    nc.vector.tensor_mul(tlog[:], d21[:], einv[:])
    nc.scalar.activation(q2[:], tlog[:], AF.Exp)
    nc.vector.tensor_scalar_add(s12[:], q2[:], 1.0)
    # p1 = 1/(1+q); p2 = q * p1
    nc.vector.reciprocal(pout[:, :, 0], s12[:])
    nc.vector.tensor_mul(pout[:, :, 1], q2[:], pout[:, :, 0])
    nc.vector.tensor_copy(iout[:], idxs[:, :, 0:2].bitcast(I32))

    nc.sync.dma_start(probs_d[win], pout[:])
    nc.sync.dma_start(idx_d[win], iout[:])


def build_nc_fp32(num_devices=NCORES, nwin=NWIN):
    NWIN_ = nwin
    nc = bacc.Bacc("TRN2", target_bir_lowering=False, debug=False, num_devices=num_devices)

    xin = nc.dram_tensor("xin", [NWIN_, 128, NKC * WIN], F32, kind="ExternalInput")
    wt = nc.dram_tensor("wt", [128, NKC, ED], F32, kind="ExternalInput")
    ct = nc.dram_tensor("ct", [128, 2, NE], F32, kind="ExternalInput")
    bt = nc.dram_tensor("bt", [128, 2], F32, kind="ExternalInput")
    one = nc.dram_tensor("one", [128, 1], F32, kind="ExternalInput")
    probs_d = nc.dram_tensor("probs", [NWIN_, 128, 2 * NG], F32, kind="ExternalOutput")
    idx_d = nc.dram_tensor("idx", [NWIN_, 128, 2 * NG], I32, kind="ExternalOutput")

    with tile.TileContext(nc) as tc:
        with (
            tc.tile_pool(name="consts", bufs=1) as consts,
            tc.tile_pool(name="xin_p", bufs=2) as xin_p,
            tc.tile_pool(name="xp_p", bufs=2) as xp_p,
            tc.tile_pool(name="xp2_p", bufs=2) as xp2_p,
            tc.tile_pool(name="sim_p", bufs=3) as sim_p,
            tc.tile_pool(name="small_p", bufs=2) as small_p,
            tc.tile_pool(name="out_p", bufs=2) as out_p,
            tc.tile_pool(name="ps_xp", bufs=4, space="PSUM") as ps_xp,
            tc.tile_pool(name="ps_sim", bufs=2, space="PSUM") as ps_sim,
            tc.tile_pool(name="ps_ss", bufs=2, space="PSUM") as ps_ss,
        ):
            wt_sb = consts.tile([128, NKC, ED], F32)
            ct_sb = consts.tile([128, 2, NE], F32)
            bt_sb = consts.tile([128, 2], F32)
            one_sb = consts.tile([128, 1], F32)
            nc.sync.dma_start(wt_sb[:], wt[:])
            nc.sync.dma_start(ct_sb[:], ct[:])
            nc.sync.dma_start(bt_sb[:], bt[:])
            nc.sync.dma_start(one_sb[:], one[:])

            for win in range(NWIN_):
                xt = xin_p.tile([128, NKC, WIN], F32, tag="xt")
                nc.sync.dma_start(xt[:], xin[win])

                xp_sb, xp2_sb = [], []
                for dh in range(2):
                    p0 = ps_xp.tile([128, 512], F32, tag="pxp")
                    p1 = ps_xp.tile([128, 512], F32, tag="pxp")
                    for k in range(NKC):
                        w_ap = wt_sb[:, k, dh * 128 : (dh + 1) * 128]
                        nc.tensor.matmul(
                            p0[:], w_ap, xt[:, k, 0:512],
                            start=(k == 0), stop=(k == NKC - 1),
                        )
                        nc.tensor.matmul(
                            p1[:], w_ap, xt[:, k, 512:1024],
                            start=(k == 0), stop=(k == NKC - 1),
                        )
                    xp = xp_p.tile([128, WIN], F32, tag=f"xp{dh}")
                    xp2 = xp2_p.tile([128, WIN], F32, tag=f"xp2{dh}")
                    bias = bt_sb[:, dh : dh + 1]
                    nc.scalar.activation(xp[:, 0:512], p0[:], AF.Identity, bias=bias)
                    nc.scalar.activation(xp[:, 512:1024], p1[:], AF.Identity, bias=bias)
                    # squares on gpsimd: ACT stays within the ln/exp table
                    # set, DVE stays free for top-k
                    nc.gpsimd.tensor_mul(xp2[:, 0:512], xp[:, 0:512], xp[:, 0:512])
                    nc.gpsimd.tensor_mul(xp2[:, 512:1024], xp[:, 512:1024], xp[:, 512:1024])
                    xp_sb.append(xp)
                    xp2_sb.append(xp2)

                def sim_maker(psim, gs):
                    for j in range(2):
                        nc.tensor.matmul(
                            psim[:], xp_sb[j][:, gs], ct_sb[:, j, :],
                            start=(j == 0), stop=(j == 1),
                        )

                _common_epilogue(
                    nc, tc, (small_p, out_p, sim_p, ps_sim, ps_ss),
                    win, xp2_sb, one_sb, sim_maker, probs_d, idx_d,
                )

    nc.compile()
    return nc


def build_nc_bf16c(num_devices=NCORES, nwin=NWIN):
    NWIN_ = nwin
    nc = bacc.Bacc("TRN2", target_bir_lowering=False, debug=False, num_devices=num_devices)

    # x ships as hi/lo bf16 pair — same bytes as fp32
    xin = nc.dram_tensor("xin", [NWIN_, 128, 2 * NKC * WIN], BF16, kind="ExternalInput")
    wt = nc.dram_tensor("wt", [128, 2, NKC, ED], BF16, kind="ExternalInput")
    ct = nc.dram_tensor("ct", [128, 2, 2, NE], BF16, kind="ExternalInput")
    bt = nc.dram_tensor("bt", [128, 2], F32, kind="ExternalInput")
    one = nc.dram_tensor("one", [128, 1], F32, kind="ExternalInput")
    probs_d = nc.dram_tensor("probs", [NWIN_, 128, 2 * NG], F32, kind="ExternalOutput")
    idx_d = nc.dram_tensor("idx", [NWIN_, 128, 2 * NG], I32, kind="ExternalOutput")

    with tile.TileContext(nc) as tc:
        with (
            tc.tile_pool(name="consts", bufs=1) as consts,
            tc.tile_pool(name="xin_p", bufs=2) as xin_p,
            tc.tile_pool(name="xp_p", bufs=2) as xp_p,
            tc.tile_pool(name="xps_p", bufs=2) as xps_p,
            tc.tile_pool(name="xp2_p", bufs=2) as xp2_p,
            tc.tile_pool(name="sim_p", bufs=3) as sim_p,
            tc.tile_pool(name="small_p", bufs=2) as small_p,
            tc.tile_pool(name="out_p", bufs=2) as out_p,
            tc.tile_pool(name="ps_xp", bufs=4, space="PSUM") as ps_xp,
            tc.tile_pool(name="ps_sim", bufs=2, space="PSUM") as ps_sim,
            tc.tile_pool(name="ps_ss", bufs=2, space="PSUM") as ps_ss,
        ):
            wt_sb = consts.tile([128, 2, NKC, ED], BF16)
            ct_sb = consts.tile([128, 2, 2, NE], BF16)
            bt_sb = consts.tile([128, 2], F32)
            one_sb = consts.tile([128, 1], F32)
            nc.sync.dma_start(wt_sb[:], wt[:])
            nc.sync.dma_start(ct_sb[:], ct[:])
            nc.sync.dma_start(bt_sb[:], bt[:])
            nc.sync.dma_start(one_sb[:], one[:])

            for win in range(NWIN_):
                # free layout: (h/l, kchunk, tok)
                xt = xin_p.tile([128, 2, NKC, WIN], BF16, tag="xt")
                nc.sync.dma_start(xt[:], xin[win])

                xph_sb, xpl_sb, xp2_sb = [], [], []
                for dh in range(2):
                    p0 = ps_xp.tile([128, 512], F32, tag="pxp")
                    p1 = ps_xp.tile([128, 512], F32, tag="pxp")
                    first, last = (0, 0), (NKC - 1, 1)
                    for k in range(NKC):
                        wh = wt_sb[:, 0, k, dh * 128 : (dh + 1) * 128]
                        wl = wt_sb[:, 1, k, dh * 128 : (dh + 1) * 128]
                        xh0, xh1 = xt[:, 0, k, 0:512], xt[:, 0, k, 512:1024]
                        xl0, xl1 = xt[:, 1, k, 0:512], xt[:, 1, k, 512:1024]
                        # weight-reuse order: Wh×(xh,xl), then Wl×xh
                        st = k == 0
                        nc.tensor.matmul(p0[:], wh, xh0, start=st, stop=False)
                        nc.tensor.matmul(p1[:], wh, xh1, start=st, stop=False)
                        nc.tensor.matmul(p0[:], wh, xl0, start=False, stop=False)
                        nc.tensor.matmul(p1[:], wh, xl1, start=False, stop=False)
                        sp = k == NKC - 1
                        nc.tensor.matmul(p0[:], wl, xh0, start=False, stop=sp)
                        nc.tensor.matmul(p1[:], wl, xh1, start=False, stop=sp)
                    xph = xp_p.tile([128, WIN], BF16, tag=f"xph{dh}")
                    xpl = xp_p.tile([128, WIN], BF16, tag=f"xpl{dh}")
                    xps = xps_p.tile([128, WIN], F32, tag=f"xps{dh}")
                    xp2 = xp2_p.tile([128, WIN], F32, tag=f"xp2{dh}")
                    bias = bt_sb[:, dh : dh + 1]
                    # hi = bf16(psum + b) on ACT; lo = (psum + b) - hi on DVE
                    nc.scalar.activation(xph[:, 0:512], p0[:], AF.Identity, bias=bias)
                    nc.scalar.activation(xph[:, 512:1024], p1[:], AF.Identity, bias=bias)
                    nc.vector.scalar_tensor_tensor(
                        xpl[:, 0:512], p0[:], bias, xph[:, 0:512],
                        op0=mybir.AluOpType.add, op1=mybir.AluOpType.subtract,
                    )
                    nc.vector.scalar_tensor_tensor(
                        xpl[:, 512:1024], p1[:], bias, xph[:, 512:1024],
                        op0=mybir.AluOpType.add, op1=mybir.AluOpType.subtract,
                    )
                    # ss operand: xp2 = (hi + lo)^2 on gpsimd (otherwise idle)
                    nc.gpsimd.tensor_add(xps[:], xph[:], xpl[:])
                    nc.gpsimd.tensor_mul(xp2[:], xps[:], xps[:])
                    xph_sb.append(xph)
                    xpl_sb.append(xpl)
                    xp2_sb.append(xp2)

                def sim_maker(psim, gs):
                    for j in range(2):
                        ch = ct_sb[:, j, 0, :]
                        cl = ct_sb[:, j, 1, :]
                        st = j == 0
                        sp = j == 1
                        nc.tensor.matmul(psim[:], xph_sb[j][:, gs], ch, start=st, stop=False)
                        nc.tensor.matmul(psim[:], xph_sb[j][:, gs], cl, start=False, stop=False)
                        nc.tensor.matmul(psim[:], xpl_sb[j][:, gs], ch, start=False, stop=sp)

                _common_epilogue(
                    nc, tc, (small_p, out_p, sim_p, ps_sim, ps_ss),
                    win, xp2_sb, one_sb, sim_maker, probs_d, idx_d,
                )

    nc.compile()
    return nc


_NC_CACHE = {}


def _get_nc():
    key = MODE
    if key not in _NC_CACHE:
        _NC_CACHE[key] = (
            build_nc_fp32() if MODE == "fp32" else build_nc_bf16c()
        )
    return _NC_CACHE[key]


def _bf16_split(a):
    hi = a.astype(BF)
    lo = (a - hi.astype(np.float32)).astype(BF)
    return hi, lo


def _host_consts(W, b, centers):
    cn = centers / np.maximum(
        np.linalg.norm(centers, axis=1, keepdims=True), 1e-12
    ).astype(np.float32)
    cT = np.ascontiguousarray(cn.astype(np.float32).T)  # [ED, NE]
    wT = np.ascontiguousarray(W.T)  # [IN_DIM, ED]
    bt_h = np.ascontiguousarray(b.reshape(2, 128).T)
    one_h = np.ones((128, 1), dtype=np.float32)
    if MODE == "fp32":
        ct_h = np.ascontiguousarray(cT.reshape(2, 128, NE).transpose(1, 0, 2))
        wt_h = np.ascontiguousarray(wT.reshape(NKC, 128, ED).transpose(1, 0, 2))
    else:
        w_hi, w_lo = _bf16_split(wT)  # [IN_DIM, ED]
        c_hi, c_lo = _bf16_split(cT)  # [ED, NE]
        # wt[p, h/l, k, e] = w_{h/l}[k*128+p, e]
        wt_h = np.ascontiguousarray(
            np.stack(
                [w_hi.reshape(NKC, 128, ED), w_lo.reshape(NKC, 128, ED)], axis=0
            ).transpose(2, 0, 1, 3)
        )
        ct_h = np.ascontiguousarray(
            np.stack(
                [c_hi.reshape(2, 128, NE), c_lo.reshape(2, 128, NE)], axis=1
            ).transpose(2, 1, 0, 3)
        )
    return wt_h, ct_h, bt_h, one_h


def _prep_core_inputs(x_flat, wt_h, ct_h, bt_h, one_h, core):
    xs = x_flat[core * TPC : (core + 1) * TPC]  # [TPC, IN_DIM]
    if MODE == "fp32":
        # xin[i, p, (c, t)] = xs[i*WIN + t, c*128 + p]
        arr = np.ascontiguousarray(
            xs.reshape(NWIN, WIN, NKC, 128).transpose(0, 3, 2, 1)
        ).reshape(NWIN, 128, NKC * WIN)
    else:
        x_hi, x_lo = _bf16_split(xs)
        # xin[i, p, (h/l, c, t)] = x_{h/l}[i*WIN + t, c*128 + p]
        arr = np.ascontiguousarray(
            np.stack(
                [
                    x_hi.reshape(NWIN, WIN, NKC, 128),
                    x_lo.reshape(NWIN, WIN, NKC, 128),
                ],
                axis=1,
            ).transpose(0, 4, 1, 3, 2)
        ).reshape(NWIN, 128, 2 * NKC * WIN)
    return {"xin": arr, "wt": wt_h, "ct": ct_h, "bt": bt_h, "one": one_h}


def kernel(x, W, b, centers, top_k, **_unused):
    assert int(top_k) == 2
    x = np.ascontiguousarray(np.asarray(x, dtype=np.float32))
    W = np.asarray(W, dtype=np.float32)
    b = np.asarray(b, dtype=np.float32)
    centers = np.asarray(centers, dtype=np.float32)

    wt_h, ct_h, bt_h, one_h = _host_consts(W, b, centers)
    x_flat = x.reshape(TOKS, IN_DIM)
    in_maps = [
        _prep_core_inputs(x_flat, wt_h, ct_h, bt_h, one_h, c) for c in range(NCORES)
    ]

    nc = _get_nc()
    res = run_bass_kernel_spmd(nc, in_maps, list(range(NCORES)))

    probs_parts = []
    idx_parts = []
    for c in range(NCORES):
        pr = res.results[c]["probs"]  # [NWIN, 128, 2*NG]
        ix = res.results[c]["idx"]
        # out[(i*WIN + g*128 + p), j] = tile[i, p, (g, j)]
        pr = pr.reshape(NWIN, 128, NG, 2).transpose(0, 2, 1, 3).reshape(TPC, 2)
        ix = ix.reshape(NWIN, 128, NG, 2).transpose(0, 2, 1, 3).reshape(TPC, 2)
        probs_parts.append(pr)
        idx_parts.append(ix)

    probs = np.concatenate(probs_parts, axis=0).reshape(BS, NTOK, 2).astype(np.float32)
    idx = np.concatenate(idx_parts, axis=0).reshape(BS, NTOK, 2).astype(np.int32)
    return probs, idx
